# revision 4
# baseline (speedup 1.0000x reference)
"""Causal multi-head self-attention kernel for Trainium2 (Bass/Tile), 8 cores.

Problem: B=4, T=2048, D=1024, H=16 (DH=64), fp32, causal mask, no padding.

Sharding (8 cores): core c = 2*b + hg handles batch b = c//2 and head-group
hg = c%2 (8 of 16 heads). Each core computes its QKV projection slice, causal
attention for its heads, and a partial output projection over its 512
features. Host sums the two partial projections per batch (exact fp32 adds,
same associativity class as the reference's single matmul accumulation).

Per-core device pipeline (all matmuls in float32r: 1 cycle/row on the PE
at moving-dim >= 256, ~1.5e-4 scale-relative rounding — validated on HW):
  ph1: x [T,D] -> xT chunks via PE transpose (contraction dim must sit on
       SBUF partitions).
  ph2: qT,kT = (W_qk)^T x^T in feature-major layout [feat, tok]; V in
       token-major layout [tok, feat] with a ones column appended per head
       (V_ext [keys, 8*65]) so the attention-value matmul also produces the
       softmax denominator.
  ph3: per (head, key-block 128, query-block 512): S^T = K Q^T -> psum;
       P^T = exp(S^T/8) (ACT, fused scale, no max subtraction needed: scores
       are ~N(0,1) so exp cannot overflow); causal masking of diagonal
       blocks via precomputed 0/1 masks; O_ext^T = V_ext^T P^T accumulated
       over key blocks -> [65, 512] (row 64 = softmax denominator l);
       normalize O^T rows by 1/l broadcast.
  ph4: y = O^T^T W_out via psum accumulation over the 4 feature chunks; DMA
       out. O^T (feature-major) is exactly the lhsT the PE needs, so no
       transposes are required anywhere past ph1.
"""
import os
import numpy as np

B, T, D, H = 4, 2048, 1024, 16
DH = 64
HL = 8            # heads per core
FL = HL * DH      # 512 local features
NCORES = 8
DC = D // 128     # 8 contraction chunks
NTB = T // 512    # 4 big token blocks
NKB = T // 128    # 16 key blocks
NQB = T // 512    # 4 query blocks
SCALE = 1.0 / 8.0  # 1/sqrt(DH)

_PROGRAM_CACHE = {}
LAST_RESULTS = None


def _build_program(is_causal: bool):
    import concourse.mybir as mybir
    import concourse.tile as tile
    from concourse import bacc

    F32 = mybir.dt.float32
    F32R = mybir.dt.float32r
    AF = mybir.ActivationFunctionType
    ALU = mybir.AluOpType

    nc = bacc.Bacc("TRN2", target_bir_lowering=False, debug=False)
    x = nc.dram_tensor("x", [T, D], F32, kind="ExternalInput").ap()
    w_qkv = nc.dram_tensor("w_qkv", [D, 3 * FL], F32, kind="ExternalInput").ap()
    w_out = nc.dram_tensor("w_out", [FL, D], F32, kind="ExternalInput").ap()
    y = nc.dram_tensor("y", [T, D], F32, kind="ExternalOutput").ap()

    with tile.TileContext(nc) as tc:
        with tc.tile_pool(name="const", bufs=1) as constp, \
             tc.tile_pool(name="qkTp", bufs=1) as qkTp, \
             tc.tile_pool(name="vextp", bufs=1) as vextp:
            identity = constp.tile([128, 128], F32)
            nc.gpsimd.memset(identity, 0.0)
            nc.gpsimd.affine_select(
                out=identity, in_=identity, compare_op=ALU.not_equal,
                fill=1.0, base=0, pattern=[[-1, 128]], channel_multiplier=1)
            ones8 = constp.tile([128, 8], F32)
            nc.gpsimd.memset(ones8, 1.0)

            # qkT[0..3]: qT features, qkT[4..7]: kT features; [feat128, T]
            qkT = [qkTp.tile([128, T], F32R, name=f"qkT{i}") for i in range(8)]
            # V_ext[kb]: [128 keys, 8 heads * (64 dims + ones col)]
            vext = [vextp.tile([128, HL * 65], F32R, name=f"vext{i}")
                    for i in range(NKB)]
            for kb in range(NKB):
                nc.vector.tensor_copy(
                    vext[kb].rearrange("p (h c) -> p h c", h=HL)[:, :, 64:65],
                    ones8.rearrange("p (h c) -> p h c", c=1))

            # ---- ph1 + ph2: transposes and projections ----
            with tc.tile_pool(name="ph2", bufs=1) as ph2p, \
                 tc.tile_pool(name="ph2s", bufs=2) as stage, \
                 tc.tile_pool(name="ps_t", bufs=2, space="PSUM") as ps_t, \
                 tc.tile_pool(name="ps_p", bufs=3, space="PSUM") as ps_p:
                wqkv_r = [ph2p.tile([128, 3 * FL], F32R, name=f"wqkvr{dc}")
                          for dc in range(DC)]
                for dc in range(DC):
                    wst = stage.tile([128, 3 * FL], F32, name="wst", tag="wst")
                    nc.sync.dma_start(wst, w_qkv[dc * 128:(dc + 1) * 128, :])
                    nc.vector.tensor_copy(wqkv_r[dc], wst)

                for tb in range(NTB):
                    xst = []
                    for ts_ in range(4):
                        xs = stage.tile([128, D], F32, name="xs", tag="xs",
                                        bufs=5)
                        t0 = (tb * 4 + ts_) * 128
                        nc.sync.dma_start(xs, x[t0:t0 + 128, :])
                        xst.append(xs)
                    xTc = [ph2p.tile([128, 512], F32R, name=f"xtc{dc}",
                                     tag=f"xtc{dc}") for dc in range(DC)]
                    for dc in range(DC):
                        pst = ps_t.tile([128, 512], F32, name="pst", tag="pst")
                        for ts_ in range(4):
                            nc.tensor.transpose(
                                pst[:, ts_ * 128:(ts_ + 1) * 128],
                                xst[ts_][:, dc * 128:(dc + 1) * 128], identity)
                        nc.vector.tensor_copy(xTc[dc], pst)
                    # q,k projections: feature-major
                    for fb in range(8):
                        pqk = ps_p.tile([128, 512], F32, name="pqk", tag="pp")
                        for dc in range(DC):
                            nc.tensor.matmul(
                                pqk, wqkv_r[dc][:, fb * 128:(fb + 1) * 128],
                                xTc[dc], start=(dc == 0), stop=(dc == DC - 1))
                        nc.vector.tensor_copy(
                            qkT[fb][:, tb * 512:(tb + 1) * 512], pqk)
                    # v projection: token-major, head-strided into vext
                    for ts_ in range(4):
                        pv = ps_p.tile([128, 512], F32, name="pv", tag="pp")
                        for dc in range(DC):
                            nc.tensor.matmul(
                                pv, xTc[dc][:, ts_ * 128:(ts_ + 1) * 128],
                                wqkv_r[dc][:, 2 * FL:3 * FL],
                                start=(dc == 0), stop=(dc == DC - 1))
                        kb = tb * 4 + ts_
                        nc.vector.tensor_copy(
                            vext[kb].rearrange("p (h c) -> p h c",
                                               h=HL)[:, :, 0:64],
                            pv.rearrange("p (h c) -> p h c", h=HL))

            # ---- ph3 + ph4 ----
            with tc.tile_pool(name="otp", bufs=1) as otp:
                # OT[0..3]: attention output, feature-major [feat128, T]
                OT = [otp.tile([128, T], F32R, name=f"OT{i}") for i in range(4)]
                masks = otp.tile([128, 4, 512], F32)
                nc.gpsimd.memset(masks, 1.0)
                for j in range(4):
                    # keep where key (128j + p) <= query f
                    nc.gpsimd.affine_select(
                        out=masks[:, j, :], in_=masks[:, j, :],
                        compare_op=ALU.is_ge, fill=0.0, base=-128 * j,
                        pattern=[[1, 512]], channel_multiplier=-1)

                with tc.tile_pool(name="ptp", bufs=3) as ptp, \
                     tc.tile_pool(name="nrm", bufs=2) as nrmp, \
                     tc.tile_pool(name="ps_st", bufs=2, space="PSUM") as ps_st, \
                     tc.tile_pool(name="ps_ot", bufs=2, space="PSUM") as ps_ot:
                    for qb in range(NQB):
                        kbs = list(range(4 * (qb + 1))) if is_causal \
                            else list(range(NKB))
                        for hp in range(4):  # head pair (2hp, 2hp+1)
                            otx = [ps_ot.tile([65, 512], F32, name=f"otx{par}",
                                              tag=f"otx{par}")
                                   for par in range(2)]
                            for i_kb, kb in enumerate(kbs):
                                st = ps_st.tile([128, 1024], F32, name="st",
                                                tag="st")
                                for par in range(2):
                                    nc.tensor.matmul(
                                        st[:, par * 512:(par + 1) * 512],
                                        qkT[4 + hp][par * 64:(par + 1) * 64,
                                                    kb * 128:(kb + 1) * 128],
                                        qkT[hp][par * 64:(par + 1) * 64,
                                                qb * 512:(qb + 1) * 512],
                                        start=True, stop=True)
                                pt = ptp.tile([128, 1024], F32R, name="pt",
                                              tag="pt")
                                nc.scalar.activation(pt, st, AF.Exp,
                                                     scale=SCALE)
                                if is_causal and kb >= 4 * qb:
                                    j = kb - 4 * qb
                                    for par in range(2):
                                        sl = slice(par * 512, (par + 1) * 512)
                                        nc.vector.tensor_mul(
                                            pt[:, sl], pt[:, sl],
                                            masks[:, j, :])
                                for par in range(2):
                                    h = 2 * hp + par
                                    nc.tensor.matmul(
                                        otx[par],
                                        vext[kb][:, h * 65:(h + 1) * 65],
                                        pt[:, par * 512:(par + 1) * 512],
                                        start=(i_kb == 0),
                                        stop=(i_kb == len(kbs) - 1))
                            for par in range(2):
                                recip = nrmp.tile([1, 512], F32, name="recip",
                                                  tag="recip")
                                nc.vector.reciprocal(recip, otx[par][64:65, :])
                                bc = nrmp.tile([64, 512], F32, name="bc",
                                               tag="bc")
                                nc.gpsimd.partition_broadcast(bc, recip)
                                nc.vector.tensor_mul(
                                    OT[hp][par * 64:(par + 1) * 64,
                                           qb * 512:(qb + 1) * 512],
                                    otx[par][0:64, :], bc)

                # ---- ph4: output projection ----
                with tc.tile_pool(name="ph4", bufs=1) as ph4p, \
                     tc.tile_pool(name="ph4s", bufs=2) as st4, \
                     tc.tile_pool(name="ysbp", bufs=3) as ysbp, \
                     tc.tile_pool(name="ps_y", bufs=4, space="PSUM") as ps_y:
                    wout_r = [ph4p.tile([128, D], F32R, name=f"woutr{fb}")
                              for fb in range(4)]
                    for fb in range(4):
                        wst4 = st4.tile([128, D], F32, name="wst4", tag="wst4")
                        nc.sync.dma_start(wst4, w_out[fb * 128:(fb + 1) * 128, :])
                        nc.vector.tensor_copy(wout_r[fb], wst4)
                    for tb in range(T // 128):
                        ysb = ysbp.tile([128, D], F32, name="ysb", tag="ysb")
                        for nb in range(2):
                            py = ps_y.tile([128, 512], F32, name="py", tag="py")
                            for fb in range(4):
                                nc.tensor.matmul(
                                    py, OT[fb][:, tb * 128:(tb + 1) * 128],
                                    wout_r[fb][:, nb * 512:(nb + 1) * 512],
                                    start=(fb == 0), stop=(fb == 3))
                            nc.scalar.copy(ysb[:, nb * 512:(nb + 1) * 512], py)
                        nc.sync.dma_start(y[tb * 128:(tb + 1) * 128, :], ysb)

    nc.compile()
    return nc


def _get_program(is_causal: bool):
    key = ("causal" if is_causal else "full")
    if key not in _PROGRAM_CACHE:
        _PROGRAM_CACHE[key] = _build_program(is_causal)
    return _PROGRAM_CACHE[key]


def _numpy_fallback(x, W_qkv, W_out, attn_mask, key_padding_mask):
    import math
    qkv = x @ W_qkv
    q, k, v = np.split(qkv, 3, axis=-1)
    q = q.reshape(B, T, H, DH).transpose(0, 2, 1, 3)
    k = k.reshape(B, T, H, DH).transpose(0, 2, 1, 3)
    v = v.reshape(B, T, H, DH).transpose(0, 2, 1, 3)
    scores = np.einsum('bhqd,bhkd->bhqk', q, k) / math.sqrt(DH)
    scores = np.where(attn_mask[None, None, :, :], -np.inf, scores)
    scores = np.where(key_padding_mask[:, None, None, :], -np.inf, scores)
    scores = scores - scores.max(axis=-1, keepdims=True)
    attn = np.exp(scores)
    attn = attn / attn.sum(axis=-1, keepdims=True)
    out = np.einsum('bhqk,bhkd->bhqd', attn, v)
    out = out.transpose(0, 2, 1, 3).reshape(B, T, D)
    return (out @ W_out).astype(np.float32)


def build_in_maps(inputs):
    x = np.ascontiguousarray(np.asarray(inputs["x"], dtype=np.float32))
    W_qkv = np.ascontiguousarray(np.asarray(inputs["W_qkv"], dtype=np.float32))
    W_out = np.ascontiguousarray(np.asarray(inputs["W_out"], dtype=np.float32))
    in_maps = []
    for c in range(NCORES):
        b, hg = c // 2, c % 2
        cols = slice(hg * FL, (hg + 1) * FL)
        w_qkv_local = np.ascontiguousarray(np.concatenate(
            [W_qkv[:, D * i:D * (i + 1)][:, cols] for i in range(3)], axis=1))
        w_out_local = np.ascontiguousarray(W_out[cols, :])
        in_maps.append({"x": x[b], "w_qkv": w_qkv_local,
                        "w_out": w_out_local})
    return in_maps


def kernel(x, W_qkv, W_out, attn_mask, key_padding_mask):
    global LAST_RESULTS
    x = np.ascontiguousarray(np.asarray(x, dtype=np.float32))
    W_qkv = np.ascontiguousarray(np.asarray(W_qkv, dtype=np.float32))
    W_out = np.ascontiguousarray(np.asarray(W_out, dtype=np.float32))
    attn_mask = np.asarray(attn_mask).astype(bool)
    key_padding_mask = np.asarray(key_padding_mask).astype(bool)

    causal = np.array_equal(
        attn_mask, np.triu(np.ones((T, T), dtype=bool), k=1))
    nomask = not attn_mask.any()
    if key_padding_mask.any() or not (causal or nomask):
        return _numpy_fallback(x, W_qkv, W_out, attn_mask, key_padding_mask)

    os.environ["BASS_NEVER_TRACE"] = "1"  # axon NTFF hook unavailable here
    from concourse.bass_utils import run_bass_kernel_spmd

    nc = _get_program(causal)
    in_maps = build_in_maps(
        {"x": x, "W_qkv": W_qkv, "W_out": W_out})

    res = run_bass_kernel_spmd(nc, in_maps, core_ids=list(range(NCORES)))
    LAST_RESULTS = res
    out = np.zeros((B, T, D), dtype=np.float32)
    for c in range(NCORES):
        out[c // 2] += res.results[c]["y"]
    return out


# revision 8
# speedup vs baseline: 1.2643x; 1.2643x over previous
"""Causal multi-head self-attention kernel for Trainium2 (Bass/Tile), 8 cores.

Problem: B=4, T=2048, D=1024, H=16 (DH=64), fp32, causal mask, no padding.

Sharding (8 cores): core c = 2*b + hg handles batch b = c//2 and head-group
hg = c%2 (8 of 16 heads). Each core computes its QKV projection slice, causal
attention for its heads, and a partial output projection over its 512
features. Host sums the two partial projections per batch (exact fp32 adds,
same associativity class as the reference's single matmul accumulation).

Per-core device pipeline (all matmuls in float32r: 1 cycle/row on the PE
at moving-dim >= 256, ~1.5e-4 scale-relative rounding — validated on HW):
  ph1: x [T,D] -> xT chunks via PE transpose (contraction dim must sit on
       SBUF partitions).
  ph2: qT,kT = (W_qk)^T x^T in feature-major layout [feat, tok]; V in
       token-major layout [tok, feat] with a ones column appended per head
       (V_ext [keys, 8*65]) so the attention-value matmul also produces the
       softmax denominator.
  ph3: per (head, key-block 128, query-block 512): S^T = K Q^T -> psum;
       P^T = exp(S^T/8) (ACT, fused scale, no max subtraction needed: scores
       are ~N(0,1) so exp cannot overflow); causal masking of diagonal
       blocks via precomputed 0/1 masks; O_ext^T = V_ext^T P^T accumulated
       over key blocks -> [65, 512] (row 64 = softmax denominator l);
       normalize O^T rows by 1/l broadcast.
  ph4: y = O^T^T W_out via psum accumulation over the 4 feature chunks; DMA
       out. O^T (feature-major) is exactly the lhsT the PE needs, so no
       transposes are required anywhere past ph1.
"""
import os
import numpy as np

B, T, D, H = 4, 2048, 1024, 16
DH = 64
HL = 8            # heads per core
FL = HL * DH      # 512 local features
NCORES = 8
DC = D // 128     # 8 contraction chunks
NTB = T // 512    # 4 big token blocks
NKB = T // 128    # 16 key blocks
NQB = T // 512    # 4 query blocks
SCALE = 1.0 / 8.0  # 1/sqrt(DH)

_PROGRAM_CACHE = {}
LAST_RESULTS = None


def _build_program(is_causal: bool):
    import concourse.mybir as mybir
    import concourse.tile as tile
    from concourse import bacc

    F32 = mybir.dt.float32
    F32R = mybir.dt.float32r
    AF = mybir.ActivationFunctionType
    ALU = mybir.AluOpType

    nc = bacc.Bacc("TRN2", target_bir_lowering=False, debug=False)
    x = nc.dram_tensor("x", [T, D], F32, kind="ExternalInput").ap()
    w_qkv = nc.dram_tensor("w_qkv", [D, 3 * FL], F32, kind="ExternalInput").ap()
    w_out = nc.dram_tensor("w_out", [FL, D], F32, kind="ExternalInput").ap()
    y = nc.dram_tensor("y", [T, D], F32, kind="ExternalOutput").ap()

    with tile.TileContext(nc) as tc:
        with tc.tile_pool(name="const", bufs=1) as constp, \
             tc.tile_pool(name="qkTp", bufs=1) as qkTp, \
             tc.tile_pool(name="vextp", bufs=1) as vextp:
            identity = constp.tile([128, 128], F32)
            nc.gpsimd.memset(identity, 0.0)
            nc.gpsimd.affine_select(
                out=identity, in_=identity, compare_op=ALU.not_equal,
                fill=1.0, base=0, pattern=[[-1, 128]], channel_multiplier=1)
            ones8 = constp.tile([128, 8], F32)
            nc.gpsimd.memset(ones8, 1.0)

            # qkT[0..3]: qT features, qkT[4..7]: kT features; [feat128, T]
            qkT = [qkTp.tile([128, T], F32R, name=f"qkT{i}") for i in range(8)]
            # V_ext[kb]: [128 keys, 8 heads * (64 dims + ones col)]
            vext = [vextp.tile([128, HL * 65], F32R, name=f"vext{i}")
                    for i in range(NKB)]
            for kb in range(NKB):
                nc.vector.tensor_copy(
                    vext[kb].rearrange("p (h c) -> p h c", h=HL)[:, :, 64:65],
                    ones8.rearrange("p (h c) -> p h c", c=1))

            # ---- ph1 + ph2: transposes and projections ----
            with tc.tile_pool(name="ph2", bufs=1) as ph2p, \
                 tc.tile_pool(name="ph2s", bufs=2) as stage, \
                 tc.tile_pool(name="ps_t", bufs=2, space="PSUM") as ps_t, \
                 tc.tile_pool(name="ps_p", bufs=3, space="PSUM") as ps_p:
                wqkv_r = [ph2p.tile([128, 3 * FL], F32R, name=f"wqkvr{dc}")
                          for dc in range(DC)]
                for dc in range(DC):
                    wst = stage.tile([128, 3 * FL], F32, name="wst", tag="wst")
                    nc.sync.dma_start(wst, w_qkv[dc * 128:(dc + 1) * 128, :])
                    nc.vector.tensor_copy(wqkv_r[dc], wst)

                for tb in range(NTB):
                    xst = []
                    for ts_ in range(4):
                        xs = stage.tile([128, D], F32, name="xs", tag="xs",
                                        bufs=5)
                        t0 = (tb * 4 + ts_) * 128
                        nc.sync.dma_start(xs, x[t0:t0 + 128, :])
                        xst.append(xs)
                    xTc = [ph2p.tile([128, 512], F32R, name=f"xtc{dc}",
                                     tag=f"xtc{dc}") for dc in range(DC)]
                    for dc in range(DC):
                        pst = ps_t.tile([128, 512], F32, name="pst", tag="pst")
                        for ts_ in range(4):
                            nc.tensor.transpose(
                                pst[:, ts_ * 128:(ts_ + 1) * 128],
                                xst[ts_][:, dc * 128:(dc + 1) * 128], identity)
                        nc.vector.tensor_copy(xTc[dc], pst)
                    # q,k projections: feature-major
                    for fb in range(8):
                        pqk = ps_p.tile([128, 512], F32, name="pqk", tag="pp")
                        for dc in range(DC):
                            nc.tensor.matmul(
                                pqk, wqkv_r[dc][:, fb * 128:(fb + 1) * 128],
                                xTc[dc], start=(dc == 0), stop=(dc == DC - 1))
                        nc.vector.tensor_copy(
                            qkT[fb][:, tb * 512:(tb + 1) * 512], pqk)
                    # v projection: token-major, head-strided into vext
                    for ts_ in range(4):
                        pv = ps_p.tile([128, 512], F32, name="pv", tag="pp")
                        for dc in range(DC):
                            nc.tensor.matmul(
                                pv, xTc[dc][:, ts_ * 128:(ts_ + 1) * 128],
                                wqkv_r[dc][:, 2 * FL:3 * FL],
                                start=(dc == 0), stop=(dc == DC - 1))
                        kb = tb * 4 + ts_
                        nc.vector.tensor_copy(
                            vext[kb].rearrange("p (h c) -> p h c",
                                               h=HL)[:, :, 0:64],
                            pv.rearrange("p (h c) -> p h c", h=HL))

            # ---- ph3 + ph4 ----
            with tc.tile_pool(name="otp", bufs=1) as otp:
                # OT[0..3]: attention output, feature-major [feat128, T]
                OT = [otp.tile([128, T], F32R, name=f"OT{i}") for i in range(4)]

                with tc.tile_pool(name="ptp", bufs=3) as ptp, \
                     tc.tile_pool(name="nrm", bufs=2) as nrmp, \
                     tc.tile_pool(name="ps_st", bufs=2, space="PSUM") as ps_st, \
                     tc.tile_pool(name="ps_ot", bufs=2, space="PSUM") as ps_ot:
                    for qb in range(NQB):
                        kbs = list(range(4 * (qb + 1))) if is_causal \
                            else list(range(NKB))
                        for hp in range(4):  # head pair (2hp, 2hp+1)
                            otx = [ps_ot.tile([65, 512], F32, name=f"otx{par}",
                                              tag=f"otx{par}")
                                   for par in range(2)]
                            for i_kb, kb in enumerate(kbs):
                                st = ps_st.tile([128, 1024], F32, name="st",
                                                tag="st")
                                # row-packed pair: K=64 each, strips (0,0)/(64,0)
                                for par in range(2):
                                    nc.tensor.matmul(
                                        st[:, par * 512:(par + 1) * 512],
                                        qkT[4 + hp][par * 64:(par + 1) * 64,
                                                    kb * 128:(kb + 1) * 128],
                                        qkT[hp][par * 64:(par + 1) * 64,
                                                qb * 512:(qb + 1) * 512],
                                        start=True, stop=True,
                                        tile_position=(par * 64, 0))
                                pt = ptp.tile([128, 1024], F32R, name="pt",
                                              tag="pt")
                                nc.scalar.activation(pt, st, AF.Exp,
                                                     scale=SCALE)
                                if is_causal and kb >= 4 * qb:
                                    j = kb - 4 * qb
                                    # zero where key 128j+p > query f (gpsimd)
                                    ptv = pt.rearrange("p (g f) -> p g f", g=2)
                                    nc.gpsimd.affine_select(
                                        out=ptv, in_=ptv,
                                        compare_op=ALU.is_ge, fill=0.0,
                                        base=-128 * j,
                                        pattern=[[0, 2], [1, 512]],
                                        channel_multiplier=-1)
                                for par in range(2):
                                    h = 2 * hp + par
                                    nc.tensor.matmul(
                                        otx[par],
                                        vext[kb][:, h * 65:(h + 1) * 65],
                                        pt[:, par * 512:(par + 1) * 512],
                                        start=(i_kb == 0),
                                        stop=(i_kb == len(kbs) - 1))
                            lrow = nrmp.tile([1, 1024], F32, name="lrow",
                                             tag="lrow")
                            for par in range(2):
                                nc.vector.tensor_copy(
                                    lrow[:, par * 512:(par + 1) * 512],
                                    otx[par][64:65, :])
                            recip = nrmp.tile([1, 1024], F32, name="recip",
                                              tag="recip")
                            nc.vector.reciprocal(recip, lrow)
                            bc = nrmp.tile([64, 1024], F32, name="bc",
                                           tag="bc")
                            nc.gpsimd.partition_broadcast(bc, recip)
                            for par in range(2):
                                nc.vector.tensor_mul(
                                    OT[hp][par * 64:(par + 1) * 64,
                                           qb * 512:(qb + 1) * 512],
                                    otx[par][0:64, :],
                                    bc[:, par * 512:(par + 1) * 512])

                # ---- ph4: output projection ----
                with tc.tile_pool(name="ph4", bufs=1) as ph4p, \
                     tc.tile_pool(name="ph4s", bufs=2) as st4, \
                     tc.tile_pool(name="ysbp", bufs=3) as ysbp, \
                     tc.tile_pool(name="ps_y", bufs=4, space="PSUM") as ps_y:
                    wout_r = [ph4p.tile([128, D], F32R, name=f"woutr{fb}")
                              for fb in range(4)]
                    for fb in range(4):
                        wst4 = st4.tile([128, D], F32, name="wst4", tag="wst4")
                        nc.sync.dma_start(wst4, w_out[fb * 128:(fb + 1) * 128, :])
                        nc.vector.tensor_copy(wout_r[fb], wst4)
                    for tb in range(T // 128):
                        ysb = ysbp.tile([128, D], F32, name="ysb", tag="ysb")
                        for nb in range(2):
                            py = ps_y.tile([128, 512], F32, name="py", tag="py")
                            for fb in range(4):
                                nc.tensor.matmul(
                                    py, OT[fb][:, tb * 128:(tb + 1) * 128],
                                    wout_r[fb][:, nb * 512:(nb + 1) * 512],
                                    start=(fb == 0), stop=(fb == 3))
                            if nb == 0:
                                nc.scalar.copy(ysb[:, 0:512], py)
                            else:
                                nc.vector.tensor_copy(ysb[:, 512:1024], py)
                        nc.sync.dma_start(y[tb * 128:(tb + 1) * 128, :], ysb)

    nc.compile()
    return nc


def _get_program(is_causal: bool):
    key = ("causal" if is_causal else "full")
    if key not in _PROGRAM_CACHE:
        _PROGRAM_CACHE[key] = _build_program(is_causal)
    return _PROGRAM_CACHE[key]


def _numpy_fallback(x, W_qkv, W_out, attn_mask, key_padding_mask):
    import math
    qkv = x @ W_qkv
    q, k, v = np.split(qkv, 3, axis=-1)
    q = q.reshape(B, T, H, DH).transpose(0, 2, 1, 3)
    k = k.reshape(B, T, H, DH).transpose(0, 2, 1, 3)
    v = v.reshape(B, T, H, DH).transpose(0, 2, 1, 3)
    scores = np.einsum('bhqd,bhkd->bhqk', q, k) / math.sqrt(DH)
    scores = np.where(attn_mask[None, None, :, :], -np.inf, scores)
    scores = np.where(key_padding_mask[:, None, None, :], -np.inf, scores)
    scores = scores - scores.max(axis=-1, keepdims=True)
    attn = np.exp(scores)
    attn = attn / attn.sum(axis=-1, keepdims=True)
    out = np.einsum('bhqk,bhkd->bhqd', attn, v)
    out = out.transpose(0, 2, 1, 3).reshape(B, T, D)
    return (out @ W_out).astype(np.float32)


def build_in_maps(inputs):
    x = np.ascontiguousarray(np.asarray(inputs["x"], dtype=np.float32))
    W_qkv = np.ascontiguousarray(np.asarray(inputs["W_qkv"], dtype=np.float32))
    W_out = np.ascontiguousarray(np.asarray(inputs["W_out"], dtype=np.float32))
    in_maps = []
    for c in range(NCORES):
        b, hg = c // 2, c % 2
        cols = slice(hg * FL, (hg + 1) * FL)
        w_qkv_local = np.ascontiguousarray(np.concatenate(
            [W_qkv[:, D * i:D * (i + 1)][:, cols] for i in range(3)], axis=1))
        w_out_local = np.ascontiguousarray(W_out[cols, :])
        in_maps.append({"x": x[b], "w_qkv": w_qkv_local,
                        "w_out": w_out_local})
    return in_maps


def kernel(x, W_qkv, W_out, attn_mask, key_padding_mask):
    global LAST_RESULTS
    x = np.ascontiguousarray(np.asarray(x, dtype=np.float32))
    W_qkv = np.ascontiguousarray(np.asarray(W_qkv, dtype=np.float32))
    W_out = np.ascontiguousarray(np.asarray(W_out, dtype=np.float32))
    attn_mask = np.asarray(attn_mask).astype(bool)
    key_padding_mask = np.asarray(key_padding_mask).astype(bool)

    causal = np.array_equal(
        attn_mask, np.triu(np.ones((T, T), dtype=bool), k=1))
    nomask = not attn_mask.any()
    if key_padding_mask.any() or not (causal or nomask):
        return _numpy_fallback(x, W_qkv, W_out, attn_mask, key_padding_mask)

    os.environ["BASS_NEVER_TRACE"] = "1"  # axon NTFF hook unavailable here
    from concourse.bass_utils import run_bass_kernel_spmd

    nc = _get_program(causal)
    in_maps = build_in_maps(
        {"x": x, "W_qkv": W_qkv, "W_out": W_out})

    res = run_bass_kernel_spmd(nc, in_maps, core_ids=list(range(NCORES)))
    LAST_RESULTS = res
    out = np.zeros((B, T, D), dtype=np.float32)
    for c in range(NCORES):
        out[c // 2] += res.results[c]["y"]
    return out
